# revision 37
# baseline (speedup 1.0000x reference)
"""MoE (top-2 of 8 experts) Trainium2 kernel.

Strategy (expert-parallel over 8 NeuronCores):
  - Router runs on host (~0.1% of FLOPs); it defines the dispatch.
  - Each core e receives the tokens routed to expert e (gathered, transposed
    to [D, C], zero-padded to capacity C) plus expert e's weights, and runs
    the 3-layer MLP on-device in a transposed dataflow:
        h1T = relu(W1^T x^T + b1)   [H,  C]
        h2T = relu(W2^T h1T + b2)   [H2, C]
        yT  = W3^T h2T + b3         [O,  C]
  - Host combines per-expert outputs with the renormalized top-2 routing
    weights (scatter-add), matching the reference's dense-combine semantics.
  - Matmuls in bf16 with fp32 PSUM accumulation.
  - Capacity C = 2184 trims padding to the measured max expert load; any
    overflow beyond C is handled by extra (small) rounds, so correctness
    never depends on C.

Perf structure (vs the 281us baseline; measured ~262us at full clock):
  - DMA schedule: each HWDGE queue moves only ~180 GB/s, so every large
    tensor is split across BOTH queues (SP=sync, ACT=scalar) in consume
    order: x0 k-halves first, then w1 m-groups alternating queues (the
    first two groups are single m-tiles so the first matmul is gated by
    fewer critical bytes), then w3 + w2 m-groups, then x1..x4.  First
    matmul fires at ~13.8us (was ~19us single-queue, with w2 queued
    behind all the x tiles); L1 of tile 0 is PE-bound from there on.
  - The pairing of x-chunks and weight-groups with queues is chosen so
    every matmul needs at most ONE new DMA-completion semaphore — the ISA
    carries a single sync wait per instruction and codegen rejects more.
  - L3 (O=10 wide) runs col-tiled: 4 concurrent 32-column PE groups (two
    accumulating matmuls each), then a 4-op DVE chain (copy + 3 adds, one
    PSUM operand per op) reduces the groups into the output tile.
  - h1/h2 live in per-chunk tiles so consumer matmuls wait only on the
    chunk they read, not the stage's last evacuation; one 1-element DVE
    fence per tile (reading the previous tile's osb) absorbs all older
    own-engine ticks so no instruction needs a second sync wait.
  - 16 full-array dummy matmuls on zeroed scratch bridge the DMA-only
    first ~15us so the PE HAM clock-gate lifts before real work.
  - Outputs leave per-tile via gpsimd SWDGE (HWDGE outputs would need a
    ring-throttle wait on top of the data wait - two sync waits).
  - Run-to-run variance: the chip sometimes runs the whole kernel at
    2.0GHz (P0 power state) - +-20% on any single measurement.
"""

import re as _re

import numpy as np
import ml_dtypes

import bass_rust as _bass_rust
import concourse.bass as bass
import concourse.mybir as mybir
import concourse.tile as tile
from concourse.bass_utils import run_bass_kernel_spmd


def _split_drain_and_barrier(self, tick_clock, wait_clock):
    """Replacement for TileContext._drain_and_barrier.

    The stock version hangs every outstanding proc semaphore wait on one
    Drain instruction; the walrus in this environment rejects any
    instruction carrying more than one sync wait. Emit the same waits as
    individual sync-engine wait_ge instructions (one wait each) before a
    clean drain instead.
    """
    ticks = [
        int(v)
        for v in _re.findall(r"\d+", repr(tick_clock.global_clock))
    ]
    for proc, sem in sorted(self.sems.allocated().items()):
        if proc < len(ticks) and ticks[proc] > 0:
            self.nc.sync.wait_ge(sem, _bass_rust.tick_to_sem(ticks[proc], proc))
    self.nc.sync.drain()

    self.nc.all_engine_barrier()
    assert self.sems is not None
    popped = self.nc._tile_sem_poison_stack.pop()
    assert popped is self._sem_poison
    self.nc.clear_and_free_semaphores(list(self.sems.allocated().values()))
    # No trailing barrier: the one above already guarantees every engine
    # is quiescent before the sem clears, and the clears are independent
    # register writes - the extra barrier only lengthens the exit tail.


tile.TileContext._drain_and_barrier = _split_drain_and_barrier

B, D, H, E, O, TOP_K = 8192, 1024, 2048, 8, 10, 2
H2 = H // 2
NCORES = 8
P = 128

TWS = [512, 512, 512, 512, 136]   # token tile widths (<=512 = one PSUM bank)
C = sum(TWS)                      # per-expert token capacity (tokens, padded)
OVERFLOW_TWS = [512]              # small NEFF for the (never-seen) case of
                                  # an expert exceeding C tokens
KD = D // P       # 8   k-chunks for layer 1
MH = H // P       # 16  m-tiles for layer 1 / k-chunks for layer 2
MH2 = H2 // P     # 8   m-tiles for layer 2 / k-chunks for layer 3

BF16 = mybir.dt.bfloat16
F32 = mybir.dt.float32
_nbf16 = ml_dtypes.bfloat16


NW1 = KD * H          # w1 columns in the packed weight tile
NW2 = MH * H2         # w2 columns
NW3 = MH2 * O         # w3 columns
W2G = 4               # w2 arrives in this many m-major group DMAs
L3T = True            # col-tiled layer 3


def _build_nc(with_bias: bool, tws) -> bass.Bass:
    cap = sum(tws)
    nc = bass.Bass()
    # Host pre-packs everything into the on-chip layout:
    #  xt   [128, KD, C]  - x gathered/transposed, k-chunks on axis 1
    #  w1/w2 packed m-major: for fixed m-tile the KD/MH k-chunk blocks are
    #  adjacent, so group g's DMA delivers complete early m-tiles first.
    #  w3 packed k-major (tiny).
    xt = nc.dram_tensor("xt", [P, KD, cap], BF16, kind="ExternalInput")
    w1d = nc.dram_tensor("w1p", [P, NW1], BF16, kind="ExternalInput")
    w2d = nc.dram_tensor("w2p", [P, NW2], BF16, kind="ExternalInput")
    w3d = nc.dram_tensor("w3p", [P, NW3], BF16, kind="ExternalInput")
    if with_bias:
        bias = nc.dram_tensor("bias", [1, H + H2 + O], BF16, kind="ExternalInput")
    out = nc.dram_tensor("out", [O, cap], F32, kind="ExternalOutput")

    relu_kw = dict(op0=mybir.AluOpType.max)

    with tile.TileContext(nc) as tc:
        with (
            tc.tile_pool(name="weights", bufs=1) as wpool,
            tc.tile_pool(name="xin", bufs=1) as xpool,
            # ps1 gets 4 banks: with 3, a matmul group periodically stalls
            # ~1.3us on its buffer's WAR against a late-running evacuation
            # (the trace shows a ~432ns issue-gap metronome from this).
            # ps3 runs fine single-banked — consecutive L3s are ~55us apart.
            tc.tile_pool(name="ps1", bufs=4, space="PSUM") as ps1pool,
            tc.tile_pool(name="ps2", bufs=3, space="PSUM") as ps2pool,
            tc.tile_pool(name="ps3", bufs=1, space="PSUM") as ps3pool,
            tc.tile_pool(name="acts", bufs=2) as apool,
        ):
            # ---- DMA schedule ----------------------------------------
            # Each HWDGE queue moves ~180 GB/s (the two together saturate
            # HBM), so every large tensor is split across BOTH queues,
            # interleaved in consume order: x0 halves first, then w1
            # groups alternating, then w2 groups, w3, then x1..x4.
            xsb_tiles = []
            off = 0
            for t, tw in enumerate(tws):
                xsb = xpool.tile([P, KD, tw], BF16, tag=f"x{t}")
                if t == 0:
                    half = KD // 2
                    nc.sync.dma_start(xsb[:, :half, :], xt[:, :half, off:off + tw])
                    nc.scalar.dma_start(xsb[:, half:, :], xt[:, half:, off:off + tw])
                xsb_tiles.append(xsb)
                off += tw

            # First two w1 groups are single m-tiles (256KB) so the very
            # first matmul is gated by ~0.75MB on its queue instead of 1MB;
            # the rest arrive as pairs, alternating queues in consume order.
            W1_GROUPS = [(0, 1), (1, 1), (2, 2), (4, 2), (6, 2),
                         (8, 2), (10, 2), (12, 2), (14, 2)]
            w1g_tiles = []
            w1_group_of = {}
            for g, (m0_, nm) in enumerate(W1_GROUPS):
                w1g = wpool.tile([P, nm * KD * P], BF16, name=f"w1g{g}")
                eng = nc.sync if g % 2 == 0 else nc.scalar
                eng.dma_start(
                    w1g, w1d[:, m0_ * KD * P:(m0_ + nm) * KD * P])
                w1g_tiles.append(w1g)
                for mm in range(m0_, m0_ + nm):
                    w1_group_of[mm] = (g, mm - m0_)

            # w3 goes out on scalar BEFORE the scalar w2 groups so that the
            # wait for any later w2 group subsumes the w3 wait (keeps the
            # L3 matmuls' waits single).
            w3sb = wpool.tile([P, NW3], BF16)
            nc.scalar.dma_start(w3sb, w3d[:, :])
            MG2 = MH2 // W2G      # m-tiles per w2 group (2)
            w2g_tiles = []
            for g in range(W2G):
                w2g = wpool.tile([P, MG2 * MH * P], BF16, name=f"w2g{g}")
                eng = nc.sync if g % 2 == 0 else nc.scalar
                eng.dma_start(
                    w2g, w2d[:, g * MG2 * MH * P:(g + 1) * MG2 * MH * P])
                w2g_tiles.append(w2g)

            off = 0
            for t, tw in enumerate(tws):
                if t > 0:
                    eng = nc.sync if t % 2 == 1 else nc.scalar
                    eng.dma_start(xsb_tiles[t], xt[:, :, off:off + tw])
                off += tw

            def w1s(k, m):
                g, mm_ = w1_group_of[m]
                off = (mm_ * KD + k) * P
                return w1g_tiles[g][:, off:off + P]

            def w2s(k, m):
                g, mm_ = divmod(m, MG2)
                off = (mm_ * MH + k) * P
                return w2g_tiles[g][:, off:off + P]

            def w3s(k):
                off = k * O
                return w3sb[:, off:off + O]

            if with_bias:
                # Bias folded into each accumulation group as one extra K=1
                # matmul against a ones row: psum[m, n] += b[m] * 1.
                bsb = wpool.tile([1, H + H2 + O], BF16)
                nc.sync.dma_start(bsb, bias[:, :])
                ones = wpool.tile([1, max(tws)], BF16)
                nc.vector.memset(ones, 1.0)

            def bias_mm(ps, lo, hi, tw, **kw):
                if with_bias:
                    nc.tensor.matmul(
                        ps, bsb[:, lo:hi], ones[:, :tw], start=False, stop=True,
                        **kw,
                    )

            # Scratch row: the warm-up dummies read it, and the per-tile
            # 1-element fence copy (see emit_l1) writes its first column.
            fence = wpool.tile([1, 4], BF16)
            nc.vector.memset(fence, 0.0)

            # HAM warm-up: full-array dummy matmuls on zeroed scratch
            # bridge the DMA-only window (~8.3us to ~15us) so the PE
            # clock-gate lifts before the real matmul stream begins.
            if len(tws) > 1:
                warm_w = wpool.tile([P, P], BF16, name="warm_w")
                warm_in = wpool.tile([P, 512], BF16, name="warm_in")
                nc.vector.memset(warm_w, 0.0)
                nc.vector.memset(warm_in, 0.0)
                warm_ps = ps1pool.tile([P, 512], F32, tag="ps1", name="warm")
                for _ in range(11):
                    nc.tensor.matmul(
                        warm_ps, warm_w, warm_in,
                        start=True, stop=True, skip_group_check=True,
                    )



            tok_offs = []
            off = 0
            for tw in tws:
                tok_offs.append(off)
                off += tw
            h1_of = {}
            osb_of = {}

            def emit_l1(t):
                tw = tws[t]
                xsb = xsb_tiles[t]
                # One fence per tile: a 1-element DVE read of the previous
                # tile's osb (the last DVE write of that tile) absorbs every
                # older own-engine WAW/WAR tick in one wait, so the per-chunk
                # activation tiles below never need a second sync wait.
                if t >= 1:
                    nc.vector.tensor_copy(fence[:, 0:1], osb_of[t - 1][0:1, 0:1])
                # Per-chunk h1 tiles: precise region deps, so L2's first
                # matmuls never wait on the last h1 evacuation.
                h1sb = [apool.tile([P, tw], BF16, tag=f"h1_{m}", name=f"h1_{m}") for m in range(MH)]
                h1_of[t] = h1sb
                for m in range(MH):
                    ps = ps1pool.tile([P, 512], F32, tag="ps1", name="ps1t")[:, :tw]
                    for k in range(KD):
                        nc.tensor.matmul(
                            ps,
                            w1s(k, m),
                            xsb[:, k, :],
                            start=(k == 0),
                            stop=(k == KD - 1) and not with_bias,
                        )
                    bias_mm(ps, m * P, (m + 1) * P, tw)
                    nc.vector.tensor_scalar(
                        h1sb[m], ps, 0.0, None, **relu_kw
                    )

            def emit_l23(t):
                tw = tws[t]
                tok = slice(tok_offs[t], tok_offs[t] + tw)
                h1sb = h1_of.pop(t)
                h2sb = [apool.tile([P, tw], BF16, tag=f"h2_{m}", name=f"h2_{m}") for m in range(MH2)]
                for m in range(MH2):
                    ps = ps2pool.tile([P, 512], F32, tag="ps2", name="ps2t")[:, :tw]
                    for k in range(MH):
                        nc.tensor.matmul(
                            ps,
                            w2s(k, m),
                            h1sb[k],
                            start=(k == 0),
                            stop=(k == MH - 1) and not with_bias,
                        )
                    bias_mm(ps, H + m * P, H + (m + 1) * P, tw)
                    nc.vector.tensor_scalar(
                        h2sb[m], ps, 0.0, None, **relu_kw
                    )

                osb = wpool.tile([O, tw], F32, name=f"osb{t}")
                osb_of[t] = osb
                if L3T:
                    # Col-tiled L3: k-chunk pairs accumulate into 4
                    # independent 32-row PE column groups, which all run
                    # concurrently; a 3-op DVE chain reduces the groups.
                    ps3 = ps3pool.tile([P, 512], F32, tag="ps3", name="ps3t")
                    for g in range(4):
                        for j in range(2):
                            k = 2 * g + j
                            nc.tensor.matmul(
                                ps3[32 * g:32 * g + O, :tw],
                                w3s(k),
                                h2sb[k],
                                start=(j == 0),
                                stop=(j == 1) and not (with_bias and g == 0),
                                tile_position=(0, 32 * g),
                                skip_group_check=True,
                            )
                    bias_mm(ps3[0:O, :tw], H + H2, H + H2 + O, tw,
                            tile_position=(0, 0), skip_group_check=True)
                    # DVE may read only ONE non-scalar input from PSUM per
                    # instruction: copy group 0 to SBUF, then chain adds
                    # that each pull one PSUM group.
                    # Per-tile temp buffers: sharing one temp across tiles
                    # makes each tile's first L3 DVE op wait on the previous
                    # tile's DVE tick — an own-engine wait on top of the PE
                    # wait, and the ISA wait slot fits only one.
                    t0 = wpool.tile([O, 512], F32, name=f"l3tmp0_{t}")[:, :tw]
                    t1 = wpool.tile([O, 512], F32, name=f"l3tmp1_{t}")[:, :tw]
                    nc.vector.tensor_copy(t0, ps3[0:O, :tw])
                    nc.vector.scalar_tensor_tensor(
                        t1, ps3[32:32 + O, :tw], 1.0, t0,
                        op0=mybir.AluOpType.mult, op1=mybir.AluOpType.add,
                    )
                    nc.vector.scalar_tensor_tensor(
                        t0, ps3[64:64 + O, :tw], 1.0, t1,
                        op0=mybir.AluOpType.mult, op1=mybir.AluOpType.add,
                    )
                    nc.vector.scalar_tensor_tensor(
                        osb, ps3[96:96 + O, :tw], 1.0, t0,
                        op0=mybir.AluOpType.mult, op1=mybir.AluOpType.add,
                    )
                else:
                    ps3 = ps3pool.tile([P, 512], F32, tag="ps3", name="ps3t")[:O, :tw]
                    for k in range(MH2):
                        nc.tensor.matmul(
                            ps3,
                            w3s(k),
                            h2sb[k],
                            start=(k == 0),
                            stop=(k == MH2 - 1) and not with_bias,
                        )
                    bias_mm(ps3, H + H2, H + H2 + O, tw)
                    nc.vector.tensor_copy(osb, ps3)
                # Per-tile SWDGE (gpsimd-issued) output transfer: overlaps
                # with later tiles' compute and keeps the HWDGE queues'
                # trigger streams single-wait (codegen rejects a second
                # sync wait on a queue that still has inputs in flight).
                nc.gpsimd.dma_start(out[:, tok], osb)

            # Sequential emission. With both queues streaming w1 and w2 in
            # interleaved groups, all weights are resident well before
            # their first use; a depth-2 software pipeline was tried and
            # buys nothing further (it also creates two-wait instructions
            # that codegen rejects).
            for t in range(len(tws)):
                emit_l1(t)
                emit_l23(t)
    return nc


_NC_CACHE: dict = {}


def _get_nc(with_bias: bool, tws) -> bass.Bass:
    key = (with_bias, tuple(tws))
    if key not in _NC_CACHE:
        _NC_CACHE[key] = _build_nc(with_bias, tws)
    return _NC_CACHE[key]


def _route(x, Wr, br):
    """Host router: softmax over logits, top-2, renormalized weights."""
    logits = x.astype(np.float32) @ Wr.astype(np.float32) + br.astype(np.float32)
    m = logits.max(axis=-1, keepdims=True)
    p = np.exp(logits - m)
    p /= p.sum(axis=-1, keepdims=True)
    top_i = np.argsort(-p, axis=-1, kind="stable")[:, :TOP_K]
    top_p = np.take_along_axis(p, top_i, axis=-1)
    top_p = top_p / top_p.sum(axis=-1, keepdims=True)
    return top_i.astype(np.int64), top_p.astype(np.float32)


def _pack_weights(W1, b1, W2, b2, W3, b3, with_bias):
    w_maps = []
    for e in range(NCORES):
        m = {
            # w1 m-major: [p, m, k, c] so the first m-groups lead the DMA
            "w1p": np.ascontiguousarray(
                W1[e].reshape(KD, P, MH, P).transpose(1, 2, 0, 3).reshape(P, NW1)
            ).astype(_nbf16),
            # w2 m-major too (delivered in m-groups on the SP queue)
            "w2p": np.ascontiguousarray(
                W2[e].reshape(MH, P, MH2, P).transpose(1, 2, 0, 3).reshape(P, NW2)
            ).astype(_nbf16),
            "w3p": np.ascontiguousarray(
                W3[e].reshape(MH2, P, O).transpose(1, 0, 2).reshape(P, NW3)
            ).astype(_nbf16),
        }
        if with_bias:
            m["bias"] = np.concatenate(
                [b1[e], b2[e], b3[e]]
            ).reshape(1, H + H2 + O).astype(_nbf16)
        w_maps.append(m)
    return w_maps


def _run_rounds(x, top_i, top_p, W1, b1, W2, b2, W3, b3, trace=False):
    """Dispatch tokens to expert-owning cores, run the NEFF, combine."""
    with_bias = bool(np.any(b1) or np.any(b2) or np.any(b3))
    w_maps = _pack_weights(W1, b1, W2, b2, W3, b3, with_bias)

    # (token, slot) pairs per expert.
    tok_by_e = []
    wt_by_e = []
    for e in range(NCORES):
        tok, slot = np.nonzero(top_i == e)
        tok_by_e.append(tok)
        wt_by_e.append(top_p[tok, slot])

    out = np.zeros((B, O), np.float32)
    offset = [0] * NCORES
    last_result = None
    first_round = True
    while True:
        active = [e for e in range(NCORES) if offset[e] < len(tok_by_e[e])]
        if not active and last_result is not None:
            break
        # Round 1 uses the full-capacity NEFF. In the (never-observed) case
        # that an expert got more than C tokens, the leftovers run through a
        # small single-tile NEFF instead of paying for a full rerun.
        tws = TWS if first_round else OVERFLOW_TWS
        cap = sum(tws)
        nc = _get_nc(with_bias, tws)
        first_round = False
        in_maps = []
        chunks = []
        for e in range(NCORES):
            tok = tok_by_e[e][offset[e]:offset[e] + cap]
            chunks.append(tok)
            xt = np.zeros((P, KD, cap), _nbf16)
            if len(tok):
                # [n, D] -> [D, n] -> k-chunks [KD, P, n] -> [P, KD, n]
                xg = x[tok].astype(_nbf16).T.reshape(KD, P, len(tok))
                xt[:, :, :len(tok)] = xg.transpose(1, 0, 2)
            in_maps.append({"xt": np.ascontiguousarray(xt), **w_maps[e]})
        res = run_bass_kernel_spmd(
            nc, in_maps, core_ids=list(range(NCORES)), trace=trace
        )
        last_result = res
        for e in range(NCORES):
            tok = chunks[e]
            if len(tok) == 0:
                continue
            y = res.results[e]["out"][:, :len(tok)].T  # [n_e, O]
            w = wt_by_e[e][offset[e]:offset[e] + len(tok)]
            np.add.at(out, tok, w[:, None] * y)
            offset[e] += len(tok)
    return out, last_result


def kernel(x, Wr, br, W1, b1, W2, b2, W3, b3):
    x = np.asarray(x, np.float32)
    top_i, top_p = _route(x, np.asarray(Wr), np.asarray(br))
    out, _ = _run_rounds(
        x, top_i, top_p,
        np.asarray(W1), np.asarray(b1), np.asarray(W2), np.asarray(b2),
        np.asarray(W3), np.asarray(b3),
    )
    return out


def run_traced(x, Wr, br, W1, b1, W2, b2, W3, b3):
    """Like kernel() but returns (out, BassKernelResults) with profile info."""
    x = np.asarray(x, np.float32)
    top_i, top_p = _route(x, np.asarray(Wr), np.asarray(br))
    return _run_rounds(
        x, top_i, top_p,
        np.asarray(W1), np.asarray(b1), np.asarray(W2), np.asarray(b2),
        np.asarray(W3), np.asarray(b3),
        trace=True,
    )


# revision 38
# speedup vs baseline: 1.0053x; 1.0053x over previous
"""MoE (top-2 of 8 experts) Trainium2 kernel.

Strategy (expert-parallel over 8 NeuronCores):
  - Router runs on host (~0.1% of FLOPs); it defines the dispatch.
  - Each core e receives the tokens routed to expert e (gathered, transposed
    to [D, C], zero-padded to capacity C) plus expert e's weights, and runs
    the 3-layer MLP on-device in a transposed dataflow:
        h1T = relu(W1^T x^T + b1)   [H,  C]
        h2T = relu(W2^T h1T + b2)   [H2, C]
        yT  = W3^T h2T + b3         [O,  C]
  - Host combines per-expert outputs with the renormalized top-2 routing
    weights (scatter-add), matching the reference's dense-combine semantics.
  - Matmuls in bf16 with fp32 PSUM accumulation.
  - Capacity C = 2184 trims padding to the measured max expert load; any
    overflow beyond C is handled by extra (small) rounds, so correctness
    never depends on C.

Perf structure (vs the 281us baseline; measured ~262us at full clock):
  - DMA schedule: each HWDGE queue moves only ~180 GB/s, so every large
    tensor is split across BOTH queues (SP=sync, ACT=scalar) in consume
    order: x0 k-halves first, then w1 m-groups alternating queues (the
    first two groups are single m-tiles so the first matmul is gated by
    fewer critical bytes), then w3 + w2 m-groups, then x1..x4.  First
    matmul fires at ~13.8us (was ~19us single-queue, with w2 queued
    behind all the x tiles); L1 of tile 0 is PE-bound from there on.
  - The pairing of x-chunks and weight-groups with queues is chosen so
    every matmul needs at most ONE new DMA-completion semaphore — the ISA
    carries a single sync wait per instruction and codegen rejects more.
  - L3 (O=10 wide) runs col-tiled: 4 concurrent 32-column PE groups (two
    accumulating matmuls each), then a 4-op DVE chain (copy + 3 adds, one
    PSUM operand per op) reduces the groups into the output tile.
  - h1/h2 live in per-chunk tiles so consumer matmuls wait only on the
    chunk they read, not the stage's last evacuation; one 1-element DVE
    fence per tile (reading the previous tile's osb) absorbs all older
    own-engine ticks so no instruction needs a second sync wait.
  - 16 full-array dummy matmuls on zeroed scratch bridge the DMA-only
    first ~15us so the PE HAM clock-gate lifts before real work.
  - Outputs leave per-tile via gpsimd SWDGE (HWDGE outputs would need a
    ring-throttle wait on top of the data wait - two sync waits).
  - Run-to-run variance: the chip sometimes runs the whole kernel at
    2.0GHz (P0 power state) - +-20% on any single measurement.
"""

import re as _re

import numpy as np
import ml_dtypes

import bass_rust as _bass_rust
import concourse.bass as bass
import concourse.mybir as mybir
import concourse.tile as tile
from concourse.bass_utils import run_bass_kernel_spmd


def _split_drain_and_barrier(self, tick_clock, wait_clock):
    """Replacement for TileContext._drain_and_barrier.

    The stock version hangs every outstanding proc semaphore wait on one
    Drain instruction; the walrus in this environment rejects any
    instruction carrying more than one sync wait. Emit the same waits as
    individual sync-engine wait_ge instructions (one wait each) before a
    clean drain instead.
    """
    ticks = [
        int(v)
        for v in _re.findall(r"\d+", repr(tick_clock.global_clock))
    ]
    for proc, sem in sorted(self.sems.allocated().items()):
        if proc < len(ticks) and ticks[proc] > 0:
            self.nc.sync.wait_ge(sem, _bass_rust.tick_to_sem(ticks[proc], proc))
    self.nc.sync.drain()

    self.nc.all_engine_barrier()
    assert self.sems is not None
    popped = self.nc._tile_sem_poison_stack.pop()
    assert popped is self._sem_poison
    self.nc.clear_and_free_semaphores(list(self.sems.allocated().values()))
    self.nc.all_engine_barrier()


tile.TileContext._drain_and_barrier = _split_drain_and_barrier

B, D, H, E, O, TOP_K = 8192, 1024, 2048, 8, 10, 2
H2 = H // 2
NCORES = 8
P = 128

TWS = [512, 512, 512, 512, 136]   # token tile widths (<=512 = one PSUM bank)
C = sum(TWS)                      # per-expert token capacity (tokens, padded)
OVERFLOW_TWS = [512]              # small NEFF for the (never-seen) case of
                                  # an expert exceeding C tokens
KD = D // P       # 8   k-chunks for layer 1
MH = H // P       # 16  m-tiles for layer 1 / k-chunks for layer 2
MH2 = H2 // P     # 8   m-tiles for layer 2 / k-chunks for layer 3

BF16 = mybir.dt.bfloat16
F32 = mybir.dt.float32
_nbf16 = ml_dtypes.bfloat16


NW1 = KD * H          # w1 columns in the packed weight tile
NW2 = MH * H2         # w2 columns
NW3 = MH2 * O         # w3 columns
W2G = 4               # w2 arrives in this many m-major group DMAs
L3T = True            # col-tiled layer 3


def _build_nc(with_bias: bool, tws) -> bass.Bass:
    cap = sum(tws)
    nc = bass.Bass()
    # Host pre-packs everything into the on-chip layout:
    #  xt   [128, KD, C]  - x gathered/transposed, k-chunks on axis 1
    #  w1/w2 packed m-major: for fixed m-tile the KD/MH k-chunk blocks are
    #  adjacent, so group g's DMA delivers complete early m-tiles first.
    #  w3 packed k-major (tiny).
    xt = nc.dram_tensor("xt", [P, KD, cap], BF16, kind="ExternalInput")
    w1d = nc.dram_tensor("w1p", [P, NW1], BF16, kind="ExternalInput")
    w2d = nc.dram_tensor("w2p", [P, NW2], BF16, kind="ExternalInput")
    w3d = nc.dram_tensor("w3p", [P, NW3], BF16, kind="ExternalInput")
    if with_bias:
        bias = nc.dram_tensor("bias", [1, H + H2 + O], BF16, kind="ExternalInput")
    out = nc.dram_tensor("out", [O, cap], F32, kind="ExternalOutput")

    relu_kw = dict(op0=mybir.AluOpType.max)

    with tile.TileContext(nc) as tc:
        with (
            tc.tile_pool(name="weights", bufs=1) as wpool,
            tc.tile_pool(name="xin", bufs=1) as xpool,
            # ps1 gets 4 banks: with 3, a matmul group periodically stalls
            # ~1.3us on its buffer's WAR against a late-running evacuation
            # (the trace shows a ~432ns issue-gap metronome from this).
            # ps3 runs fine single-banked — consecutive L3s are ~55us apart.
            tc.tile_pool(name="ps1", bufs=4, space="PSUM") as ps1pool,
            tc.tile_pool(name="ps2", bufs=3, space="PSUM") as ps2pool,
            tc.tile_pool(name="ps3", bufs=1, space="PSUM") as ps3pool,
            tc.tile_pool(name="acts", bufs=2) as apool,
        ):
            # ---- DMA schedule ----------------------------------------
            # Each HWDGE queue moves ~180 GB/s (the two together saturate
            # HBM), so every large tensor is split across BOTH queues,
            # interleaved in consume order: x0 halves first, then w1
            # groups alternating, then w2 groups, w3, then x1..x4.
            xsb_tiles = []
            off = 0
            for t, tw in enumerate(tws):
                xsb = xpool.tile([P, KD, tw], BF16, tag=f"x{t}")
                if t == 0:
                    half = KD // 2
                    nc.sync.dma_start(xsb[:, :half, :], xt[:, :half, off:off + tw])
                    nc.scalar.dma_start(xsb[:, half:, :], xt[:, half:, off:off + tw])
                xsb_tiles.append(xsb)
                off += tw

            # First two w1 groups are single m-tiles (256KB) so the very
            # first matmul is gated by ~0.75MB on its queue instead of 1MB;
            # the rest arrive as pairs, alternating queues in consume order.
            W1_GROUPS = [(0, 1), (1, 1), (2, 2), (4, 2), (6, 2),
                         (8, 2), (10, 2), (12, 2), (14, 2)]
            w1g_tiles = []
            w1_group_of = {}
            for g, (m0_, nm) in enumerate(W1_GROUPS):
                w1g = wpool.tile([P, nm * KD * P], BF16, name=f"w1g{g}")
                eng = nc.sync if g % 2 == 0 else nc.scalar
                eng.dma_start(
                    w1g, w1d[:, m0_ * KD * P:(m0_ + nm) * KD * P])
                w1g_tiles.append(w1g)
                for mm in range(m0_, m0_ + nm):
                    w1_group_of[mm] = (g, mm - m0_)

            # w3 goes out on scalar BEFORE the scalar w2 groups so that the
            # wait for any later w2 group subsumes the w3 wait (keeps the
            # L3 matmuls' waits single).
            w3sb = wpool.tile([P, NW3], BF16)
            nc.scalar.dma_start(w3sb, w3d[:, :])
            MG2 = MH2 // W2G      # m-tiles per w2 group (2)
            w2g_tiles = []
            for g in range(W2G):
                w2g = wpool.tile([P, MG2 * MH * P], BF16, name=f"w2g{g}")
                eng = nc.sync if g % 2 == 0 else nc.scalar
                eng.dma_start(
                    w2g, w2d[:, g * MG2 * MH * P:(g + 1) * MG2 * MH * P])
                w2g_tiles.append(w2g)

            off = 0
            for t, tw in enumerate(tws):
                if t > 0:
                    eng = nc.sync if t % 2 == 1 else nc.scalar
                    eng.dma_start(xsb_tiles[t], xt[:, :, off:off + tw])
                off += tw

            def w1s(k, m):
                g, mm_ = w1_group_of[m]
                off = (mm_ * KD + k) * P
                return w1g_tiles[g][:, off:off + P]

            def w2s(k, m):
                g, mm_ = divmod(m, MG2)
                off = (mm_ * MH + k) * P
                return w2g_tiles[g][:, off:off + P]

            def w3s(k):
                off = k * O
                return w3sb[:, off:off + O]

            if with_bias:
                # Bias folded into each accumulation group as one extra K=1
                # matmul against a ones row: psum[m, n] += b[m] * 1.
                bsb = wpool.tile([1, H + H2 + O], BF16)
                nc.sync.dma_start(bsb, bias[:, :])
                ones = wpool.tile([1, max(tws)], BF16)
                nc.vector.memset(ones, 1.0)

            def bias_mm(ps, lo, hi, tw, **kw):
                if with_bias:
                    nc.tensor.matmul(
                        ps, bsb[:, lo:hi], ones[:, :tw], start=False, stop=True,
                        **kw,
                    )

            # Scratch row: the warm-up dummies read it, and the per-tile
            # 1-element fence copy (see emit_l1) writes its first column.
            fence = wpool.tile([1, 4], BF16)
            nc.vector.memset(fence, 0.0)

            # HAM warm-up: full-array dummy matmuls on zeroed scratch
            # bridge the DMA-only window (~8.3us to ~15us) so the PE
            # clock-gate lifts before the real matmul stream begins.
            if len(tws) > 1:
                warm_w = wpool.tile([P, P], BF16, name="warm_w")
                warm_in = wpool.tile([P, 512], BF16, name="warm_in")
                nc.vector.memset(warm_w, 0.0)
                nc.vector.memset(warm_in, 0.0)
                warm_ps = ps1pool.tile([P, 512], F32, tag="ps1", name="warm")
                for _ in range(11):
                    nc.tensor.matmul(
                        warm_ps, warm_w, warm_in,
                        start=True, stop=True, skip_group_check=True,
                    )



            tok_offs = []
            off = 0
            for tw in tws:
                tok_offs.append(off)
                off += tw
            h1_of = {}
            osb_of = {}

            def emit_l1(t):
                tw = tws[t]
                xsb = xsb_tiles[t]
                # One fence per tile: a 1-element DVE read of the previous
                # tile's osb (the last DVE write of that tile) absorbs every
                # older own-engine WAW/WAR tick in one wait, so the per-chunk
                # activation tiles below never need a second sync wait.
                if t >= 1:
                    nc.vector.tensor_copy(fence[:, 0:1], osb_of[t - 1][0:1, 0:1])
                # Per-chunk h1 tiles: precise region deps, so L2's first
                # matmuls never wait on the last h1 evacuation.
                h1sb = [apool.tile([P, tw], BF16, tag=f"h1_{m}", name=f"h1_{m}") for m in range(MH)]
                h1_of[t] = h1sb
                for m in range(MH):
                    ps = ps1pool.tile([P, 512], F32, tag="ps1", name="ps1t")[:, :tw]
                    for k in range(KD):
                        nc.tensor.matmul(
                            ps,
                            w1s(k, m),
                            xsb[:, k, :],
                            start=(k == 0),
                            stop=(k == KD - 1) and not with_bias,
                        )
                    bias_mm(ps, m * P, (m + 1) * P, tw)
                    nc.vector.tensor_scalar(
                        h1sb[m], ps, 0.0, None, **relu_kw
                    )

            def emit_l23(t):
                tw = tws[t]
                tok = slice(tok_offs[t], tok_offs[t] + tw)
                h1sb = h1_of.pop(t)
                h2sb = [apool.tile([P, tw], BF16, tag=f"h2_{m}", name=f"h2_{m}") for m in range(MH2)]
                for m in range(MH2):
                    ps = ps2pool.tile([P, 512], F32, tag="ps2", name="ps2t")[:, :tw]
                    for k in range(MH):
                        nc.tensor.matmul(
                            ps,
                            w2s(k, m),
                            h1sb[k],
                            start=(k == 0),
                            stop=(k == MH - 1) and not with_bias,
                        )
                    bias_mm(ps, H + m * P, H + (m + 1) * P, tw)
                    nc.vector.tensor_scalar(
                        h2sb[m], ps, 0.0, None, **relu_kw
                    )

                osb = wpool.tile([O, tw], F32, name=f"osb{t}")
                osb_of[t] = osb
                if L3T:
                    # Col-tiled L3: k-chunk pairs accumulate into 4
                    # independent 32-row PE column groups, which all run
                    # concurrently; a 3-op DVE chain reduces the groups.
                    ps3 = ps3pool.tile([P, 512], F32, tag="ps3", name="ps3t")
                    for g in range(4):
                        for j in range(2):
                            k = 2 * g + j
                            nc.tensor.matmul(
                                ps3[32 * g:32 * g + O, :tw],
                                w3s(k),
                                h2sb[k],
                                start=(j == 0),
                                stop=(j == 1) and not (with_bias and g == 0),
                                tile_position=(0, 32 * g),
                                skip_group_check=True,
                            )
                    bias_mm(ps3[0:O, :tw], H + H2, H + H2 + O, tw,
                            tile_position=(0, 0), skip_group_check=True)
                    # DVE may read only ONE non-scalar input from PSUM per
                    # instruction: copy group 0 to SBUF, then chain adds
                    # that each pull one PSUM group.
                    # Per-tile temp buffers: sharing one temp across tiles
                    # makes each tile's first L3 DVE op wait on the previous
                    # tile's DVE tick — an own-engine wait on top of the PE
                    # wait, and the ISA wait slot fits only one.
                    t0 = wpool.tile([O, 512], F32, name=f"l3tmp0_{t}")[:, :tw]
                    t1 = wpool.tile([O, 512], F32, name=f"l3tmp1_{t}")[:, :tw]
                    nc.vector.tensor_copy(t0, ps3[0:O, :tw])
                    nc.vector.scalar_tensor_tensor(
                        t1, ps3[32:32 + O, :tw], 1.0, t0,
                        op0=mybir.AluOpType.mult, op1=mybir.AluOpType.add,
                    )
                    nc.vector.scalar_tensor_tensor(
                        t0, ps3[64:64 + O, :tw], 1.0, t1,
                        op0=mybir.AluOpType.mult, op1=mybir.AluOpType.add,
                    )
                    nc.vector.scalar_tensor_tensor(
                        osb, ps3[96:96 + O, :tw], 1.0, t0,
                        op0=mybir.AluOpType.mult, op1=mybir.AluOpType.add,
                    )
                else:
                    ps3 = ps3pool.tile([P, 512], F32, tag="ps3", name="ps3t")[:O, :tw]
                    for k in range(MH2):
                        nc.tensor.matmul(
                            ps3,
                            w3s(k),
                            h2sb[k],
                            start=(k == 0),
                            stop=(k == MH2 - 1) and not with_bias,
                        )
                    bias_mm(ps3, H + H2, H + H2 + O, tw)
                    nc.vector.tensor_copy(osb, ps3)
                # Per-tile SWDGE (gpsimd-issued) output transfer: overlaps
                # with later tiles' compute and keeps the HWDGE queues'
                # trigger streams single-wait (codegen rejects a second
                # sync wait on a queue that still has inputs in flight).
                nc.gpsimd.dma_start(out[:, tok], osb)

            # Sequential emission. With both queues streaming w1 and w2 in
            # interleaved groups, all weights are resident well before
            # their first use; a depth-2 software pipeline was tried and
            # buys nothing further (it also creates two-wait instructions
            # that codegen rejects).
            for t in range(len(tws)):
                emit_l1(t)
                emit_l23(t)
    return nc


_NC_CACHE: dict = {}


def _get_nc(with_bias: bool, tws) -> bass.Bass:
    key = (with_bias, tuple(tws))
    if key not in _NC_CACHE:
        _NC_CACHE[key] = _build_nc(with_bias, tws)
    return _NC_CACHE[key]


def _route(x, Wr, br):
    """Host router: softmax over logits, top-2, renormalized weights."""
    logits = x.astype(np.float32) @ Wr.astype(np.float32) + br.astype(np.float32)
    m = logits.max(axis=-1, keepdims=True)
    p = np.exp(logits - m)
    p /= p.sum(axis=-1, keepdims=True)
    top_i = np.argsort(-p, axis=-1, kind="stable")[:, :TOP_K]
    top_p = np.take_along_axis(p, top_i, axis=-1)
    top_p = top_p / top_p.sum(axis=-1, keepdims=True)
    return top_i.astype(np.int64), top_p.astype(np.float32)


def _pack_weights(W1, b1, W2, b2, W3, b3, with_bias):
    w_maps = []
    for e in range(NCORES):
        m = {
            # w1 m-major: [p, m, k, c] so the first m-groups lead the DMA
            "w1p": np.ascontiguousarray(
                W1[e].reshape(KD, P, MH, P).transpose(1, 2, 0, 3).reshape(P, NW1)
            ).astype(_nbf16),
            # w2 m-major too (delivered in m-groups on the SP queue)
            "w2p": np.ascontiguousarray(
                W2[e].reshape(MH, P, MH2, P).transpose(1, 2, 0, 3).reshape(P, NW2)
            ).astype(_nbf16),
            "w3p": np.ascontiguousarray(
                W3[e].reshape(MH2, P, O).transpose(1, 0, 2).reshape(P, NW3)
            ).astype(_nbf16),
        }
        if with_bias:
            m["bias"] = np.concatenate(
                [b1[e], b2[e], b3[e]]
            ).reshape(1, H + H2 + O).astype(_nbf16)
        w_maps.append(m)
    return w_maps


def _run_rounds(x, top_i, top_p, W1, b1, W2, b2, W3, b3, trace=False):
    """Dispatch tokens to expert-owning cores, run the NEFF, combine."""
    with_bias = bool(np.any(b1) or np.any(b2) or np.any(b3))
    w_maps = _pack_weights(W1, b1, W2, b2, W3, b3, with_bias)

    # (token, slot) pairs per expert.
    tok_by_e = []
    wt_by_e = []
    for e in range(NCORES):
        tok, slot = np.nonzero(top_i == e)
        tok_by_e.append(tok)
        wt_by_e.append(top_p[tok, slot])

    out = np.zeros((B, O), np.float32)
    offset = [0] * NCORES
    last_result = None
    first_round = True
    while True:
        active = [e for e in range(NCORES) if offset[e] < len(tok_by_e[e])]
        if not active and last_result is not None:
            break
        # Round 1 uses the full-capacity NEFF. In the (never-observed) case
        # that an expert got more than C tokens, the leftovers run through a
        # small single-tile NEFF instead of paying for a full rerun.
        tws = TWS if first_round else OVERFLOW_TWS
        cap = sum(tws)
        nc = _get_nc(with_bias, tws)
        first_round = False
        in_maps = []
        chunks = []
        for e in range(NCORES):
            tok = tok_by_e[e][offset[e]:offset[e] + cap]
            chunks.append(tok)
            xt = np.zeros((P, KD, cap), _nbf16)
            if len(tok):
                # [n, D] -> [D, n] -> k-chunks [KD, P, n] -> [P, KD, n]
                xg = x[tok].astype(_nbf16).T.reshape(KD, P, len(tok))
                xt[:, :, :len(tok)] = xg.transpose(1, 0, 2)
            in_maps.append({"xt": np.ascontiguousarray(xt), **w_maps[e]})
        res = run_bass_kernel_spmd(
            nc, in_maps, core_ids=list(range(NCORES)), trace=trace
        )
        last_result = res
        for e in range(NCORES):
            tok = chunks[e]
            if len(tok) == 0:
                continue
            y = res.results[e]["out"][:, :len(tok)].T  # [n_e, O]
            w = wt_by_e[e][offset[e]:offset[e] + len(tok)]
            np.add.at(out, tok, w[:, None] * y)
            offset[e] += len(tok)
    return out, last_result


def kernel(x, Wr, br, W1, b1, W2, b2, W3, b3):
    x = np.asarray(x, np.float32)
    top_i, top_p = _route(x, np.asarray(Wr), np.asarray(br))
    out, _ = _run_rounds(
        x, top_i, top_p,
        np.asarray(W1), np.asarray(b1), np.asarray(W2), np.asarray(b2),
        np.asarray(W3), np.asarray(b3),
    )
    return out


def run_traced(x, Wr, br, W1, b1, W2, b2, W3, b3):
    """Like kernel() but returns (out, BassKernelResults) with profile info."""
    x = np.asarray(x, np.float32)
    top_i, top_p = _route(x, np.asarray(Wr), np.asarray(br))
    return _run_rounds(
        x, top_i, top_p,
        np.asarray(W1), np.asarray(b1), np.asarray(W2), np.asarray(b2),
        np.asarray(W3), np.asarray(b3),
        trace=True,
    )
